# revision 3
# baseline (speedup 1.0000x reference)
"""Causal self-attention (B=4, T=2048, C=1024, H=16) on 8 trn2 NeuronCores.

Sharding: 8 shards = (batch b in 0..3) x (half-of-heads hh in 0..1).
Each core computes its batch's Q/K/V projections for its 8 heads, causal
flash-style attention (scores -> exp -> AV with the es-stationary 65-column
trick), and a partial out-projection; the host sums the two half-head
partials per batch and adds the folded biases.

On top of the bf16 baseline:

  - Q/K/V projections run as 3-term dual-residual fp8e4 matmuls in
    MatmulPerfMode.DoubleRow (0.5 PE cycles per output column, two
    contraction subtiles per pass):
        x @ W ~ (x8 + xr8) @ W8 + x8 @ Wr8     (xr8 @ Wr8 ~ 1e-6, dropped)
    which cuts the projection PE time to 6/8 of bf16 at ~bf16 accuracy
    (Q/K apply the Wr8 correction to only the first half of c_in - the
    uncorrected deterministic weight-quant error costs ~0.8e-2 rel err
    and saves another PE cycle per column; V keeps the full correction).
    Static weights are pre-scaled by 32 before fp8 quantization (keeps W8
    and the residual Wr8 out of e4m3's subnormal range); the 32x factor
    rides through Q/K into the exp scale (/1024) and through V/Z via a
    32.0 ones-column, so no extra evacuation work is needed. The host
    sends x8/xr8 interleaved as [C, 2, T] fp8 and W8/Wr8 separately.
  - Projection PSUM evacuations are single full-width DVE ops emitted
    after the accumulation-group stop; scores stay bf16 (fp8 Q/K storage
    costs ~1.7e-2 rel err - over the budget).
  - exp runs on ACT, except a spread of k-tiles in the ACT-bound back
    half (qc2: 3/12, qc3: 6-7/16 per unit) which run on DVE as a
    Schraudolph exp-by-bitcast: u16 = trunc(A*s + B) reinterpreted as
    bf16 approximates exp(s) to ~3%, which the softmax normalization
    mostly cancels (measured end-to-end ~4e-3 rel err).
  - The final out-projection's y DMAs rotate across the sync/scalar/
    gpsimd queues so the epilogue drain is not serialized on one HWDGE.
"""

import os
import sys

for _p in ("/opt/trn_rl_repo",):
    if _p not in sys.path and os.path.isdir(_p):
        sys.path.insert(0, _p)

from contextlib import ExitStack

import ml_dtypes
import numpy as np

import concourse.bacc as bacc
import concourse.mybir as mybir
import concourse.tile as tile
from concourse import bass_utils

B, T, C, H = 4, 2048, 1024, 16
D = 64                 # head dim
HLOC = 8               # heads per core
CS = HLOC * D          # 512: per-core slice of C on the head axis
SCALE = 1.0 / 8.0      # 1/sqrt(D)
NP = 128               # partitions
QC = 512               # q chunk (PSUM bank width in fp32)
NQC = T // QC          # 4
NKT = T // NP          # 16 k tiles
NCT = C // NP          # 8 contraction tiles for projections
NPAIR = HLOC // 2      # 4 head pairs

F32 = mybir.dt.float32
BF16 = mybir.dt.bfloat16
U16 = mybir.dt.uint16
F8 = mybir.dt.float8e4
DRM = mybir.MatmulPerfMode.DoubleRow
BF = ml_dtypes.bfloat16
F8NP = ml_dtypes.float8_e4m3
F8FN = ml_dtypes.float8_e4m3fn  # transport label for PJRT (same 8-bit payload)
NJP = NCT // 2         # 4 DoubleRow c_in-pair passes
WS = 32.0              # static weight pre-scale before fp8

_CACHE = {}


def _build():
    nc = bacc.Bacc(
        "TRN2",
        target_bir_lowering=False,
        debug=False,
        enable_asserts=False,
        num_devices=8,
    )

    qT_d = nc.dram_tensor("qT", [C, 2, T], F8, kind="ExternalInput").ap()
    kT_d = nc.dram_tensor("kT", [C, 2, T], F8, kind="ExternalInput").ap()
    vT_d = nc.dram_tensor("vT", [C, 2, T], F8, kind="ExternalInput").ap()
    Wq_d = nc.dram_tensor("Wq", [C, CS], F8, kind="ExternalInput").ap()
    Wk_d = nc.dram_tensor("Wk", [C, CS], F8, kind="ExternalInput").ap()
    Wv_d = nc.dram_tensor("Wv", [C, CS], F8, kind="ExternalInput").ap()
    Wqr_d = nc.dram_tensor("Wqr", [C // 2, CS], F8, kind="ExternalInput").ap()
    Wkr_d = nc.dram_tensor("Wkr", [C // 2, CS], F8, kind="ExternalInput").ap()
    Wvr_d = nc.dram_tensor("Wvr", [C, CS], F8, kind="ExternalInput").ap()
    Wo_d = nc.dram_tensor("Wo", [CS, C], BF16, kind="ExternalInput").ap()
    bq_d = nc.dram_tensor("bq", [CS], F32, kind="ExternalInput").ap()
    bk_d = nc.dram_tensor("bk", [CS], F32, kind="ExternalInput").ap()
    y_d = nc.dram_tensor("y", [T, C], BF16, kind="ExternalOutput").ap()

    with tile.TileContext(nc) as tc, ExitStack() as ctx:
        wpool = ctx.enter_context(tc.tile_pool(name="wpool", bufs=1))
        cpool = ctx.enter_context(tc.tile_pool(name="cpool", bufs=1))
        xpool = ctx.enter_context(tc.tile_pool(name="xpool", bufs=2))
        epool = ctx.enter_context(tc.tile_pool(name="epool", bufs=1))
        spool = ctx.enter_context(tc.tile_pool(name="spool", bufs=2))
        ypool = ctx.enter_context(tc.tile_pool(name="ypool", bufs=6))
        psP = ctx.enter_context(tc.tile_pool(name="psP", bufs=2, space="PSUM"))
        psS = ctx.enter_context(tc.tile_pool(name="psS", bufs=2, space="PSUM"))
        psO = ctx.enter_context(tc.tile_pool(name="psO", bufs=1, space="PSUM"))

        # ---- persistent weights / consts ----
        # Weights live in single wide tiles; each load is ONE batched DMA
        # ([1024, n] DRAM -> [128, 8*n] SBUF via a 3D access pattern) to
        # amortize the per-DMA HWDGE overhead. DMA priority order: Wq + qT
        # chunk 0 first so the Q projection can start as early as possible.
        def w_tile_and_dma(name, dram, n_ct, width, dt, halves=1):
            t = wpool.tile([NP, n_ct * width], dt, name=name, tag=name)
            h = n_ct // halves
            for i in range(halves):
                nc.sync.dma_start(
                    t[:, i * h * width : (i + 1) * h * width].rearrange(
                        "p (j n) -> p j n", n=width
                    ),
                    dram[i * h * NP : (i + 1) * h * NP, :].rearrange(
                        "(j p) n -> p j n", p=NP
                    ),
                )
            return t

        # x-stream tiles: one [128, 4096] tile per (input, chunk), loaded in
        # one DMA; bufs=2 per tag keeps two chunks in flight
        xcur = {}

        def emit_x_dma(c, inputs=(0, 1, 2), halves=1):
            for ii in inputs:
                xcur[(c, ii)] = xpool.tile(
                    [NP, NCT * 2 * QC], F8, name=f"x{ii}", tag=f"x{ii}"
                )
            for w in range(2):  # x8 planes for all inputs first, then residuals
                for ii in inputs:
                    x_plane_dma(c, ii, w)

        # interleave W/x half-DMAs so the first projection matmuls can start
        # after ~1MB instead of ~2MB has landed (subtile deps gate per-half)
        def w_half_dma(t, dram, width, i, parts=2):
            h = NCT // parts
            nc.sync.dma_start(
                t[:, i * h * width : (i + 1) * h * width].rearrange(
                    "p (j n) -> p j n", n=width
                ),
                dram[i * h * NP : (i + 1) * h * NP, :].rearrange(
                    "(j p) n -> p j n", p=NP
                ),
            )

        Wq_sb = wpool.tile([NP, NCT * CS], F8, name="Wq", tag="Wq")
        Wk_sb = wpool.tile([NP, NCT * CS], F8, name="Wk", tag="Wk")
        xcur[(0, 0)] = xpool.tile([NP, NCT * 2 * QC], F8, name="x0", tag="x0")
        xcur[(0, 1)] = xpool.tile([NP, NCT * 2 * QC], F8, name="x1", tag="x1")
        bq_sb = cpool.tile([NP, 4], F32, name="bq_sb", tag="bq_sb")
        bk_sb = cpool.tile([NP, 4], F32, name="bk_sb", tag="bk_sb")

        # x plane DMA: w=0 is x8, w=1 is the residual (needed only from pass B)
        def x_plane_dma(c, ii, w, i=0, parts=1, eng=None):
            x_d = (qT_d, kT_d, vT_d)[ii]
            h = NCT // parts
            (eng or nc.sync).dma_start(
                xcur[(c, ii)][:, i * h * 2 * QC : (i + 1) * h * 2 * QC].rearrange(
                    "p (j w n) -> p j w n", w=2, n=QC
                )[:, :, w, :],
                x_d[i * h * NP : (i + 1) * h * NP, w, c * QC : (c + 1) * QC]
                .rearrange("(j p) t -> p j t", p=NP),
            )

        # Prologue DMAs fan out over three queues: sync carries the W8/Wr
        # chain, scalar the x8 planes, gpsimd the residual planes + biases.
        # Arrival order matches the chunk-0 phase order (A01, A23, B+C).
        w_half_dma(Wq_sb, Wq_d, CS, 0)
        x_plane_dma(0, 0, 0, 0, parts=2, eng=nc.scalar)
        x_plane_dma(0, 0, 1, 0, parts=2, eng=nc.gpsimd)
        w_half_dma(Wq_sb, Wq_d, CS, 1)
        x_plane_dma(0, 0, 0, 1, parts=2, eng=nc.scalar)
        x_plane_dma(0, 0, 1, 1, parts=2, eng=nc.gpsimd)
        Wqr_sb = w_tile_and_dma("Wqr", Wqr_d, NCT // 2, CS, F8)
        nc.gpsimd.dma_start(bq_sb[:], bq_d.rearrange("(t p) -> p t", p=NP))
        w_half_dma(Wk_sb, Wk_d, CS, 0)
        x_plane_dma(0, 1, 0, 0, parts=2, eng=nc.scalar)
        x_plane_dma(0, 1, 1, 0, parts=2, eng=nc.gpsimd)
        w_half_dma(Wk_sb, Wk_d, CS, 1)
        x_plane_dma(0, 1, 0, 1, parts=2, eng=nc.scalar)
        x_plane_dma(0, 1, 1, 1, parts=2, eng=nc.gpsimd)
        Wkr_sb = w_tile_and_dma("Wkr", Wkr_d, NCT // 2, CS, F8)
        nc.gpsimd.dma_start(bk_sb[:], bk_d.rearrange("(t p) -> p t", p=NP))
        Wv_sb = w_tile_and_dma("Wv", Wv_d, NCT, CS, F8)
        xcur[(0, 2)] = xpool.tile([NP, NCT * 2 * QC], F8, name="x2", tag="x2")
        x_plane_dma(0, 2, 0, eng=nc.scalar)
        x_plane_dma(0, 2, 1, eng=nc.gpsimd)
        Wvr_sb = w_tile_and_dma("Wvr", Wvr_d, NCT, CS, F8)
        Wo_sb = w_tile_and_dma("Wo", Wo_d, NPAIR, C, BF16)

        # Causal mask for the single partially-masked [128,128] block of each
        # diagonal tile (the staircase: columns below the block are skipped
        # entirely, columns above are fully valid). Duplicated in two halves
        # so both heads mask with one op: mask[p, f%128] = 1.0 if f%128 >= p.
        mask_sb = cpool.tile([NP, 2 * NP], BF16, name="mask_sb", tag="mask_sb")
        nc.gpsimd.memset(mask_sb[:], 1.0)
        nc.gpsimd.affine_select(
            out=mask_sb.rearrange("p (h f) -> p h f", h=2),
            in_=mask_sb.rearrange("p (h f) -> p h f", h=2),
            pattern=[[0, 2], [1, NP]],
            compare_op=mybir.AluOpType.is_ge,
            fill=0.0,
            base=0,
            channel_multiplier=-1,
        )

        # identity for PE-side transposes (final pair's ON)
        id_sb = cpool.tile([NP, NP], BF16, name="id_sb", tag="id_sb")
        nc.gpsimd.memset(id_sb[:], 1.0)
        nc.gpsimd.affine_select(
            out=id_sb[:],
            in_=id_sb[:],
            pattern=[[1, NP]],
            compare_op=mybir.AluOpType.is_equal,
            fill=0.0,
            base=0,
            channel_multiplier=-1,
        )

        # persistent activations
        QT_sb = [
            cpool.tile([NP, T], BF16, name=f"QT{p}", tag=f"QT{p}") for p in range(NPAIR)
        ]
        KT_sb = [
            cpool.tile([NP, T], BF16, name=f"KT{p}", tag=f"KT{p}") for p in range(NPAIR)
        ]
        ON_sb = [
            cpool.tile([NP, T], BF16, name=f"ON{p}", tag=f"ON{p}") for p in range(NPAIR)
        ]
        # dedicated tile for the LAST pair's final-chunk ON: a fresh tensor
        # has no earlier writers on any DMA queue, so its transpose (on the
        # by-then-idle ACT queue) dispatches with no cross-queue WAW wait
        ON3f = cpool.tile([NP, QC], BF16, name="ON3f", tag="ON3f")
        # V' padded: per k-tile [128, 8 heads * 65], col 64 of each 65-block = 1.0
        V_sb = [
            cpool.tile([NP, HLOC * 65], BF16, name=f"V{t}", tag=f"V{t}")
            for t in range(NKT)
        ]
        for t in range(NKT):
            v3 = V_sb[t].rearrange("p (h e) -> p h e", e=65)
            nc.any.memset(v3[:, :, 64:65], WS)

        # ---------------- emission helpers ----------------
        def qk_unit_mms(qc, W8, Wr, b_sb, OUT, ii, ot):
            """DoubleRow Q/K projection for (qc, ot) as three 4-pass terms:
            A: x8 @ W8, B: xr8 @ W8, C: x8 @ Wr8 (c_in tile pairs each).
            tch-major: the 12 matmuls of each 256-col half then its evac, so
            the next unit's PSUM WAR window shrinks. Returns 6 phases
            (A01, A23, B+C+evac) x tch; evacs run on the idle Pool engine."""
            ps = psP.tile([NP, QC], F32, name="ps_proj", tag="ps_proj")
            W83 = W8.rearrange("p (j n) -> p j n", n=CS)
            Wr3 = Wr.rearrange("p (j n) -> p j n", n=CS)
            osl = slice(ot * NP, (ot + 1) * NP)
            phases = []
            for tch in range(2):
                tsl = slice(tch * 256, (tch + 1) * 256)

                def mk(stat3, wplane, i, start, stop, tsl=tsl):
                    def mm():
                        nc.tensor.matmul(
                            ps[:, tsl],
                            stat3[:, 2 * i : 2 * i + 2, osl],
                            xcur[(qc, ii)].rearrange(
                                "p (j w n) -> p j w n", w=2, n=QC
                            )[:, 2 * i : 2 * i + 2, wplane : wplane + 1, tsl],
                            start=start,
                            stop=stop,
                            perf_mode=DRM,
                        )

                    return mm

                phA = [
                    mk(W83, 0, i, start=(tch == 0 and i == 0), stop=False)
                    for i in range(NJP)
                ]
                phBC = [mk(W83, 1, i, start=False, stop=False) for i in range(NJP)]
                # W-residual correction on the first half of c_in only: the
                # uncorrected half costs ~0.9e-2 rel err (deterministic, scales
                # linearly with the uncorrected fraction) and saves 1 of 6
                # DoubleRow cycles per output column
                phBC += [
                    mk(Wr3, 0, i, start=False, stop=(tch == 1 and i == 1))
                    for i in range(2)
                ]

                if tch == 1:
                    def evac():
                        nc.vector.tensor_scalar_add(
                            OUT[ot][:, qc * QC : (qc + 1) * QC],
                            ps[:],
                            b_sb[:, ot : ot + 1],
                        )

                    last = phBC[-1]

                    def last_with_evac(last=last, evac=evac):
                        last()
                        evac()

                    phBC[-1] = last_with_evac
                phases.extend([phA[:2], phA[2:], phBC])
            return phases

        QK_ARGS = (
            (Wq_sb, Wqr_sb, bq_sb, QT_sb, 0),
            (Wk_sb, Wkr_sb, bk_sb, KT_sb, 1),
        )

        def gen_qk_proj_c0(W8, Wr, b_sb, OUT, ii, ots):
            """Chunk-0 Q/K projection for an ot pair, phase-major across both
            ots, matching the DMA arrival order (W8/x8 halves, then residual
            planes and Wr)."""
            units = {ot: qk_unit_mms(0, W8, Wr, b_sb, OUT, ii, ot) for ot in ots}
            for ph in range(6):
                for ot in ots:
                    yield from units[ot][ph]

        def gen_qk_proj_part(qc, ot):
            """Q^T and K^T DoubleRow projections for chunk qc, c_out tile ot."""
            for W8, Wr, b_sb, OUT, ii in QK_ARGS:
                for ph_mms in qk_unit_mms(qc, W8, Wr, b_sb, OUT, ii, ot):
                    yield from ph_mms

        def gen_v_proj_part(qc, ts):
            """DoubleRow V projection for chunk qc, t-subtile ts (stationary
            is the x side; moving is W). nch-major with per-half Pool evacs."""
            t_tile = qc * 4 + ts
            ps = psP.tile([NP, QC], F32, name="ps_proj", tag="ps_proj")
            Wv3 = Wv_sb.rearrange("p (j n) -> p j n", n=CS)
            Wvr3 = Wvr_sb.rearrange("p (j n) -> p j n", n=CS)
            tsl = slice(ts * NP, (ts + 1) * NP)
            for nch in range(2):
                nsl = slice(nch * 256, (nch + 1) * 256)

                def mkv(mov3, wplane, i, start, stop, nsl=nsl):
                    def mm():
                        nc.tensor.matmul(
                            ps[:, nsl],
                            xcur[(qc, 2)].rearrange(
                                "p (j w n) -> p j w n", w=2, n=QC
                            )[:, 2 * i : 2 * i + 2, wplane : wplane + 1, tsl],
                            mov3[:, 2 * i : 2 * i + 2, nsl],
                            start=start,
                            stop=stop,
                            perf_mode=DRM,
                        )

                    return mm

                mms = [
                    mkv(Wv3, 0, i, start=(nch == 0 and i == 0), stop=False)
                    for i in range(NJP)
                ]
                mms += [mkv(Wv3, 1, i, start=False, stop=False) for i in range(NJP)]
                mms += [
                    mkv(Wvr3, 0, i, start=False, stop=(nch == 1 and i == NJP - 1))
                    for i in range(NJP)
                ]

                if nch == 1:
                    def evac(t_tile=t_tile):
                        dst = V_sb[t_tile].rearrange("p (h e) -> p h e", e=65)[
                            :, :, 0:64
                        ]
                        src = ps.rearrange("p (h d) -> p h d", d=D)
                        nc.vector.tensor_copy(dst, src)

                    last = mms[-1]

                    def last_with_evac(last=last, evac=evac):
                        last()
                        evac()

                    mms[-1] = last_with_evac
                yield from mms

        def gen_out_proj_part(
            tc_, p, dma_split=False, act_copy=False, pool=None, direct_dma=False
        ):
            """Out-projection for t_tile 4*tc_+p, both 512-wide n chunks.

            act_copy: evacuate PSUM via the ACT engine instead of DVE. Used in
            the epilogue, where the in-order DVE queue is blocked behind the
            final normalization chain — DVE-side copies would stall PSUM slot
            recycling (and thus the PE) on work that is otherwise ready.
            """
            tt = 4 * tc_ + p
            tsl = slice(tt * NP, (tt + 1) * NP)
            ysb = ypool.tile([NP, C], BF16, name="ysb", tag="ysb")
            for nck in range(2):
                nsl = slice(nck * QC, (nck + 1) * QC)
                po = pool or psP
                if po is psS:
                    # scores banks are free by epilogue time: borrow them so
                    # four out-proj groups can be in flight at once
                    ps = psS.tile([NP, 2 * QC], F32, name="S2", tag="S2")[:, 0:QC]
                elif po is psO:
                    ps = psO.tile([NP, 2 * QC], F32, name="O2", tag="O")[:, 0:QC]
                else:
                    ps = po.tile([NP, QC], F32, name="ps_proj", tag="ps_proj")
                for pair in range(NPAIR):
                    last = pair == NPAIR - 1

                    def mm(ps=ps, pair=pair, last=last, tsl=tsl, nsl=nsl, nck=nck):
                        if pair == NPAIR - 1 and tc_ == NQC - 1:
                            on_src = ON3f[:, tsl.start - (NQC - 1) * QC :
                                          tsl.stop - (NQC - 1) * QC]
                        else:
                            on_src = ON_sb[pair][:, tsl]
                        nc.tensor.matmul(
                            ps[:],
                            on_src,
                            Wo_sb[:, pair * C + nsl.start : pair * C + nsl.stop],
                            start=(pair == 0),
                            stop=last,
                        )
                        if last:
                            # dma_split (windowed quads): alternate ACT/DVE so
                            # consecutive windows' PSUM slots recycle via
                            # independent engine queues
                            if act_copy and (nck == 0 or not dma_split):
                                nc.scalar.copy(ysb[:, nsl], ps[:])
                            else:
                                nc.vector.tensor_copy(ysb[:, nsl], ps[:])
                            dq = (nc.sync, nc.scalar, nc.gpsimd)[
                                (tsl.start // NP) % 3
                            ]
                            if dma_split:
                                dq.dma_start(y_d[tsl, nsl], ysb[:, nsl])
                            elif nck == 1:
                                dq.dma_start(y_d[tsl, :], ysb[:])

                    yield mm

        # ---------------- filler queues ----------------
        # proj_q: ordered projection work for chunks 0..3, drained just-in-time
        # before the attention unit that needs it, or spliced early between
        # QK/AV matmuls to keep the PE busy while ACT computes exp.
        # op_q: out-projection work, gated per chunk (eligible once the
        # chunk's attention is fully normalized); spliced into late units
        # where projection filler has run out.
        proj_q = []    # items: (chunk, closure)
        mark_qk = {}   # (qc, pair) -> proj_q index that must be drained first
        mark_av = {}   # qc -> proj_q index that must be drained before AV

        # chunk 0: j-major ot-pair order matching half-DMA arrival
        for W88, Wr, b_sb, OUT, ii in QK_ARGS:
            proj_q.extend(
                (0, f) for f in gen_qk_proj_c0(W88, Wr, b_sb, OUT, ii, (0, 1))
            )
            if ii == 1:
                mark_qk[(0, 0)] = mark_qk[(0, 1)] = len(proj_q)
            proj_q.extend(
                (0, f) for f in gen_qk_proj_c0(W88, Wr, b_sb, OUT, ii, (2, 3))
            )
            if ii == 1:
                mark_qk[(0, 2)] = mark_qk[(0, 3)] = len(proj_q)
        for ts in range(NPAIR):
            proj_q.extend((0, f) for f in gen_v_proj_part(0, ts))
        mark_av[0] = len(proj_q)

        for c in range(1, NQC):
            def dma_c(c=c):
                emit_x_dma(c)

            proj_q.append((c, dma_c))
            for p in range(NPAIR):
                proj_q.extend((c, f) for f in gen_qk_proj_part(c, p))
                mark_qk[(c, p)] = len(proj_q)
                if p == 0:
                    for ts in range(NPAIR):
                        proj_q.extend((c, f) for f in gen_v_proj_part(c, ts))
                    mark_av[c] = len(proj_q)

        op_q = []      # eligible out-proj closures (appended as chunks finish)

        state = {"pq": 0, "qc": 0}

        def drain_to(idx):
            while state["pq"] < idx:
                proj_q[state["pq"]][1]()
                state["pq"] += 1

        def splice(n):
            # pop projection filler, but never front-run more than one chunk
            # ahead of the current attention chunk (preserves filler for the
            # ACT-bound final chunk)
            k = 0
            while (
                k < n
                and state["pq"] < len(proj_q)
                and proj_q[state["pq"]][0] <= state["qc"] + 1
            ):
                proj_q[state["pq"]][1]()
                state["pq"] += 1
                k += 1
            if k == 0 and op_q:
                # ration out-proj filler: none before qc2, 1-of-3 slots in
                # qc2, free in the ACT-bound final chunk
                state["tick"] = state.get("tick", 0) + 1
                if state["qc"] == NQC - 1 or (
                    state["qc"] == NQC - 2 and state["tick"] % 3 == 0
                ):
                    op_q.pop(0)()

        # ---------------- attention with interleaved filler ----------------
        SCHED = [(qc, pair) for qc in range(NQC) for pair in range(NPAIR)]
        done = [0] * NQC
        for qc, pair in SCHED:
            if True:
                state["qc"] = qc
                kmax = 4 * (qc + 1)
                qsl = slice(qc * QC, (qc + 1) * QC)
                drain_to(mark_qk[(qc, pair)])
                es = []
                for kt in range(kmax):
                    # diagonal tiles (kt >= 4*qc) only need the q-suffix
                    # [off, 512): columns below are fully causal-masked
                    off = max(0, (kt - 4 * qc) * NP)
                    ksl = slice(kt * NP, (kt + 1) * NP)
                    S2 = psS.tile([NP, 2 * QC], F32, name="S2", tag="S2")
                    for hp in range(2):
                        psl = slice(hp * 64, (hp + 1) * 64)
                        nc.tensor.matmul(
                            S2[:, hp * QC + off : (hp + 1) * QC],
                            KT_sb[pair][psl, ksl],
                            QT_sb[pair][psl, qc * QC + off : (qc + 1) * QC],
                            start=True,
                            stop=True,
                            tile_position=(hp * 64, 0),
                        )
                    # es tiles are uint16; bf16 reads go through a bitcast
                    # view. exp runs on ACT, or - for a share of the k-tiles
                    # in the ACT-bound back half - as a Schraudolph
                    # exp-by-bitcast on DVE/Pool:
                    #   u16 = trunc(A*s + B), bf16(u16) ~ exp(s_real)
                    # (A = 128*log2e*scale, B = 128*(127-0.043677) + 0.5 for
                    # the truncating conversion; |rel err| <~ 3%, which the
                    # softmax normalization mostly cancels).
                    e2u = epool.tile(
                        [NP, 2 * QC], U16, name=f"e{kt}", tag=f"e{kt}"
                    )
                    e2 = e2u[:].bitcast(BF16)
                    s3 = S2.rearrange("p (h f) -> p h f", h=2)[:, :, off:]
                    e3 = e2.rearrange("p (h f) -> p h f", h=2)[:, :, off:]
                    # Schraudolph offload to DVE for a spread of non-diagonal
                    # k-tiles in the ACT-bound back half (Pool cannot read
                    # PSUM, so DVE is the only alternative exp engine)
                    if qc == 3 and kt in (
                        (1, 3, 5, 7, 9, 11, 13) if pair >= 2 else (1, 3, 5, 7, 9, 11)
                    ):
                        exp_eng = nc.vector
                    elif qc == 2 and kt in (1, 4, 7):
                        exp_eng = nc.vector
                    else:
                        exp_eng = None
                    if exp_eng is None:
                        nc.scalar.activation(
                            e3, s3, mybir.ActivationFunctionType.Exp,
                            scale=SCALE / (WS * WS),
                        )
                    else:
                        u3 = e2u.rearrange("p (h f) -> p h f", h=2)[:, :, off:]
                        exp_eng.tensor_scalar(
                            u3,
                            s3,
                            128.0 * 1.4426950408889634 * SCALE / (WS * WS),
                            128.0 * (127.0 - 0.043677) + 0.5,
                            mybir.AluOpType.mult,
                            mybir.AluOpType.add,
                        )
                    if off or kt == 4 * qc:  # diagonal: mask the partial block
                        eb = e2.rearrange("p (h f) -> p h f", h=2)[
                            :, :, off : off + NP
                        ]
                        nc.vector.tensor_mul(
                            eb, eb, mask_sb.rearrange("p (h f) -> p h f", h=2)
                        )
                    es.append(e2)
                    # splice filler every 2nd kt (matches the 2-tile S2
                    # pipeline depth) to halve PE array row-mode switches
                    if kt % 2 == 1 or kt == kmax - 1:
                        splice(9)
                drain_to(mark_av[qc])
                O2 = psO.tile([NP, 2 * QC], F32, name="O2", tag="O")
                oh = O2[:].rearrange("p (h x) -> p h x", h=2)
                o3 = oh[:, :, 0 : 4 * 65].rearrange("p h (s e) -> p h s e", e=65)
                for kt in range(kmax):
                    smin = max(0, kt - 4 * qc)
                    for hp in range(2):
                        h = pair * 2 + hp
                        for sub in range(smin, 4):
                            nc.tensor.matmul(
                                o3[:, hp, sub, :],
                                es[kt][
                                    :, hp * QC + sub * NP : hp * QC + (sub + 1) * NP
                                ],
                                V_sb[kt][:, h * 65 : h * 65 + 65],
                                start=(kt == 0 and sub == 0),
                                stop=(kt == 4 * qc + sub),
                                skip_group_check=True,
                            )
                        splice(3)
                zinv = spool.tile([NP, 8], F32, name="zinv", tag="zinv")
                z3 = zinv[:].rearrange("p (h s o) -> p h s o", h=2, o=1)
                nc.vector.reciprocal(z3, o3[:, :, :, 64:65])
                ONq = spool.tile([NP, QC], BF16, name="ONq", tag="ONq")
                onq4 = ONq[:].rearrange("p (s h e) -> p h s e", s=4, h=2)
                nc.vector.tensor_mul(
                    onq4,
                    o3[:, :, :, 0:64],
                    z3[:, :, :, 0:1].to_broadcast((NP, 2, 4, 64)),
                )
                # one XBAR transpose covers all four 128x128 sub-blocks (3D
                # out AP = per-sub transposes). The very last pair's ON is
                # transposed on the PE instead (53ns each, runs the moment
                # the norm-mul lands) so the epilogue's closing matmuls skip
                # the DMA queue/semaphore machinery entirely.
                if qc == NQC - 1 and pair >= NPAIR - 2:
                    psT = psO.tile([NP, 2 * QC], BF16, name="OT", tag="O")
                    for sub in range(4):
                        nc.tensor.transpose(
                            psT[:, sub * NP : (sub + 1) * NP],
                            ONq[:, sub * NP : (sub + 1) * NP],
                            id_sb[:],
                        )
                    dst = ON3f[:] if pair == NPAIR - 1 else ON_sb[pair][:, qsl]
                    nc.vector.tensor_copy(dst, psT[:, 0 : 4 * NP])
                else:
                    nc.sync.dma_start_transpose(
                        ON_sb[pair][:, qsl].rearrange("p (s q) -> p s q", q=NP),
                        ONq[:],
                    )
                done[qc] += 1
                if done[qc] == NPAIR and qc < NQC - 1:
                    # this chunk's ON is complete: its out-proj becomes eligible.
                    # Hold back two chunk-2 parts as an epilogue reserve: they
                    # depend only on chunk-2 data, so the PE can run them while
                    # the very last normalization chain completes.
                    held = (2, 3) if qc == 2 else ()
                    for p in range(NPAIR):
                        if p not in held or qc != 2:
                            op_q.extend(gen_out_proj_part(qc, p))
                    if qc == 2:
                        reserve = [
                            f
                            for p in held
                            for f in gen_out_proj_part(qc, p, act_copy=True)
                        ]

        # ---------------- epilogue ----------------
        drain_to(len(proj_q))
        while op_q:
            op_q.pop(0)()
        # Final chunk's out-projection in waves of FOUR psum groups (2 from
        # psP + 2 borrowed from the now-idle psS scores pool): emit each
        # wave's pair-0..2 matmuls (independent of the final normalization),
        # splice reserve filler while the last pair's transposes land, then
        # close with the pair-3 matmuls.
        groups = [
            list(
                gen_out_proj_part(
                    NQC - 1,
                    p,
                    dma_split=True,
                    act_copy=True,
                    pool=(psP if p % 2 == 0 else psS),
                )
            )
            for p in range(NPAIR)
        ]
        # each gen yields 8 mms = 2 psum groups of 4 (nck 0 and 1)
        quads = [g[i : i + 4] for g in groups for i in (0, 4)]
        for f in reserve:
            f()
        for w in range(0, len(quads), 4):
            wave = quads[w : w + 4]
            for g in wave:
                for mm in g[:3]:
                    mm()
            for g in wave:
                g[3]()

    nc.compile()
    return nc


def get_nc():
    if "nc" not in _CACHE:
        _CACHE["nc"] = _build()
    return _CACHE["nc"]


def _wsplit(W):
    """scaled dual fp8 of a static weight: W*WS ~ W8 + Wr. Bits are TRN
    e4m3 (ml_dtypes.float8_e4m3, max 240); the arrays are relabeled
    e4m3fn for the PJRT transport, which copies raw bytes."""
    Ws = np.asarray(W, np.float32) * WS
    W8 = Ws.astype(F8NP)
    Wr = (Ws - W8.astype(np.float32)).astype(F8NP)
    return W8.view(F8FN), Wr.view(F8FN)


def _xsplit(xT):
    """activation dual fp8, interleaved [C, 2, T] (TRN e4m3 bits,
    e4m3fn transport label)."""
    x8 = xT.astype(F8NP)
    xr = (xT - x8.astype(np.float32)).astype(F8NP)
    return np.ascontiguousarray(np.stack([x8, xr], axis=1)).view(F8FN)


def make_in_maps(k, v, q, Wq, bq, Wk, bk, Wv, bv, Wo, bo):
    k = np.asarray(k, np.float32)
    v = np.asarray(v, np.float32)
    q = np.asarray(q, np.float32)
    Wo = np.asarray(Wo, np.float32).astype(BF)
    bq = np.asarray(bq, np.float32)
    bk = np.asarray(bk, np.float32)

    in_maps = []
    for core in range(8):
        b, hh = core // 2, core % 2
        sl = slice(hh * CS, (hh + 1) * CS)
        Wq8, Wqr = _wsplit(np.asarray(Wq, np.float32)[:, sl])
        Wk8, Wkr = _wsplit(np.asarray(Wk, np.float32)[:, sl])
        Wv8, Wvr = _wsplit(np.asarray(Wv, np.float32)[:, sl])
        in_maps.append(
            {
                "qT": _xsplit(q[b].T),
                "kT": _xsplit(k[b].T),
                "vT": _xsplit(v[b].T),
                "Wq": np.ascontiguousarray(Wq8),
                "Wk": np.ascontiguousarray(Wk8),
                "Wv": np.ascontiguousarray(Wv8),
                "Wqr": np.ascontiguousarray(Wqr[: C // 2]),
                "Wkr": np.ascontiguousarray(Wkr[: C // 2]),
                "Wvr": np.ascontiguousarray(Wvr),
                "Wo": np.ascontiguousarray(Wo[sl, :]),
                "bq": np.ascontiguousarray(bq[sl] * WS),
                "bk": np.ascontiguousarray(bk[sl] * WS),
            }
        )
    return in_maps


def kernel(k, v, q, Wq, bq, Wk, bk, Wv, bv, Wo, bo):
    nc = get_nc()
    in_maps = make_in_maps(k, v, q, Wq, bq, Wk, bk, Wv, bv, Wo, bo)
    res = bass_utils.run_bass_kernel_spmd(nc, in_maps, core_ids=list(range(8)))
    # softmax rows sum to 1, so the V bias passes through attention as a
    # constant: y += bv @ Wo. Fold it into the host-side bias add.
    bias = np.asarray(bo, np.float32) + np.asarray(bv, np.float32) @ np.asarray(
        Wo, np.float32
    )
    out = np.empty((B, T, C), np.float32)
    for b in range(B):
        out[b] = (
            res.results[2 * b]["y"].astype(np.float32)
            + res.results[2 * b + 1]["y"].astype(np.float32)
            + bias
        )
    return out

